# revision 41
# baseline (speedup 1.0000x reference)
"""Bass/Tile kernel for a 3-layer bidirectional LSTM classifier on 8 TRN2 cores.

Problem shapes (hardcoded): x [256, 512, 16], H=256, 3 BiLSTM layers, fc -> [256].

Strategy: data-parallel over batch (B=32 per core, no collectives). Per core,
each layer runs as concurrent "streams" whose per-step chains hide each other's
latency: layers 0/1 use (fwd, rev); layer 2 (fwd only feeds the output) splits
the batch into two half-streams. All state is transposed (h [H, B], gates
[4H, B]) so matmuls keep weights PE-stationary; gate rows are pre-permuted to
[i, f, o, g] chunk order.

v7 numerics: every sigmoid becomes tanh via sigma(x) = (tanh(x/2)+1)/2 — the
i,f,o gate rows are prescaled 0.5 host-side so ONE tanh activation covers all 8
gate chunks; the cell update is 4 fused scalar_tensor_tensor ops on DVE.
Hidden state is stored as 2h and cell as 2c, absorbed by scaling whh / next
wih by 0.5 (fc absorbs it in numpy).

Projection: layer 0 (K=16, bias as a 17th ones-row of x) is fused directly
into the per-step PSUM accumulation group as 8 K=17 matmuls — no separate
projection phase. Layers 1/2 compute xp one 16-step block ahead at N=512 into
PSUM (one ATOMIC group per gate chunk — groups must stay per-region
sequential), then a bias-folding copy moves PSUM -> SBUF bf16, split into two
halves alternating Vector/Scalar so no single copy blocks a chain op long.
The per-step group holds the xp injection (identity matmul) plus the two
recurrent matmuls per gate chunk; activations read gates straight from PSUM."""

import os
from contextlib import ExitStack

import numpy as np
import ml_dtypes

import concourse.bass as bass
import concourse.mybir as mybir
import concourse.tile as tile
from concourse import bacc, bass_utils
from concourse.bass import ds

f32 = mybir.dt.float32
bf16 = mybir.dt.bfloat16
AF = mybir.ActivationFunctionType
AOP = mybir.AluOpType
np_bf16 = ml_dtypes.bfloat16

H = 256
NCORES = 8
BFULL = 256
TFULL = 512
I0 = 16

# gate chunk order i,i,f,f,o,o,g,g (PyTorch order in weights is i,f,g,o)
_PERM = np.concatenate(
    [np.arange(0, 512), np.arange(768, 1024), np.arange(512, 768)]
)
_MORD = (6, 7, 0, 1, 2, 3, 4, 5)
# tanh-trick prescale: i,f,o chunks 0.5 (perm order i,i,f,f,o,o,g,g)
_GATE_SCALE = np.repeat([0.5, 0.5, 0.5, 1.0], 256)[:, None]


def _prep_wih(w, half=False):
    """[1024, Din] -> stationary layout [min(Din,128), nk*1024], chunk (k, m)
    at cols k*1024 + m*128; lhsT[kk, m*128+mm] = w_perm[m*128+mm, k*128+kk]."""
    wr = np.asarray(w, np.float32)[_PERM] * _GATE_SCALE
    if half:
        wr = wr * 0.5
    din = wr.shape[1]
    if din <= 128:
        return np.ascontiguousarray(wr.T.astype(np_bf16))
    nk = din // 128
    out = np.empty((128, nk * 1024), np_bf16)
    for k in range(nk):
        out[:, k * 1024 : (k + 1) * 1024] = wr[:, k * 128 : (k + 1) * 128].T
    return out


def _prep_wih0(w, b):
    """Layer-0 stationary: [128, 1024] = [wih.T ; bias row ; zero pad] — K is
    padded to 128 so the fused matmuls pipeline with the K=128 recurrent ones
    (the x slab rows 17..127 are zeroed on device)."""
    wr = np.asarray(w, np.float32)[_PERM] * _GATE_SCALE  # [1024, 16]
    br = np.asarray(b, np.float32)[_PERM] * _GATE_SCALE[:, 0]  # [1024]
    out = np.zeros((128, 1024), np.float32)
    out[:I0] = wr.T
    out[I0] = br
    return np.ascontiguousarray(out.astype(np_bf16))


def _prep_b(b):
    bs = np.asarray(b, np.float32)[_PERM] * _GATE_SCALE[:, 0]
    return np.ascontiguousarray(bs.reshape(8, 128).T)


class _Stream:
    __slots__ = ("sid", "d", "b0", "bw")

    def __init__(self, sid, d, b0, bw):
        self.sid, self.d, self.b0, self.bw = sid, d, b0, bw


def build(nc, T=TFULL, B=32):
    """Emit the full per-core program into nc (a Bacc)."""
    TB = T * B
    U = 16
    NBLK = T // U
    UB = U * B

    xT0 = nc.dram_tensor("xT0", [I0 + 1, TB], bf16, kind="ExternalInput").ap()
    win = {}
    for l in range(3):
        for d, dn in enumerate("fr"):
            if l == 0:
                win[(l, d, "wih")] = nc.dram_tensor(
                    f"wih{l}{dn}", [128, 1024], bf16, kind="ExternalInput"
                ).ap()
            else:
                win[(l, d, "wih")] = nc.dram_tensor(
                    f"wih{l}{dn}", [128, 4096], bf16, kind="ExternalInput"
                ).ap()
                win[(l, d, "b")] = nc.dram_tensor(
                    f"b{l}{dn}", [128, 8], f32, kind="ExternalInput"
                ).ap()
            win[(l, d, "whh")] = nc.dram_tensor(
                f"whh{l}{dn}", [128, 2048], bf16, kind="ExternalInput"
            ).ap()
    b2rb = nc.dram_tensor("b2rb", [128, 8 * B], f32, kind="ExternalInput").ap()
    identin = nc.dram_tensor("ident", [128, 128], bf16, kind="ExternalInput").ap()
    out_h2f = nc.dram_tensor("h2f", [128, 2 * B], f32, kind="ExternalOutput").ap()
    out_h2r = nc.dram_tensor("h2r", [128, 2 * B], f32, kind="ExternalOutput").ap()

    with tile.TileContext(nc) as tc, ExitStack() as ctx:
        dram = ctx.enter_context(tc.tile_pool(name="dram", bufs=1, space="DRAM"))
        wpool = ctx.enter_context(tc.tile_pool(name="wts", bufs=1))
        slabpool = ctx.enter_context(tc.tile_pool(name="slab", bufs=1))
        pj = [
            ctx.enter_context(tc.tile_pool(name=f"pj{d}", bufs=2, space="PSUM"))
            for d in range(2)
        ]
        rpsB = [
            ctx.enter_context(tc.tile_pool(name=f"rps{d}", bufs=2, space="PSUM"))
            for d in range(2)
        ]
        gpool = ctx.enter_context(tc.tile_pool(name="g", bufs=4))
        state = ctx.enter_context(tc.tile_pool(name="st", bufs=1))
        tmp = ctx.enter_context(tc.tile_pool(name="tmp", bufs=4))

        xin = {
            1: dram.tile([4, 128, TB], bf16, tag="xin1", name="xin1"),
            2: dram.tile([4, 128, TB], bf16, tag="xin2", name="xin2"),
        }
        ident = wpool.tile([128, 128], bf16, tag="ident")
        nc.sync.dma_start(ident[:], identin[:])

        def load_weights(l):
            kp = 128
            kch = 1 if l == 0 else 4
            wt = {}
            for d in range(2):
                wih_t = wpool.tile([kp, kch * 1024], bf16, tag=f"wih{d}")
                nc.sync.dma_start(wih_t[:], win[(l, d, "wih")][:])
                whh_t = wpool.tile([128, 2048], bf16, tag=f"whh{d}")
                nc.sync.dma_start(whh_t[:], win[(l, d, "whh")][:])
                if l == 0:
                    b_t = None
                else:
                    b_t = wpool.tile([128, 8], f32, tag=f"b{d}")
                    nc.sync.dma_start(b_t[:], win[(l, d, "b")][:])
                wt[d] = (wih_t, whh_t, b_t)
            return wt

        def load_slab_set(l, jb, par, dirs):
            """Stage the input columns block jb needs (p=0 fwd block jb,
            p=1 the mirrored block NBLK-1-jb; layers >=1 always need both:
            the k=2,3 input channels are stored time-reversed)."""
            sl = {}
            kk = 1 if l == 0 else 4
            kp = I0 + 1 if l == 0 else 128
            for k in range(kk):
                for p in range(2):
                    t_ = slabpool.tile([kp, UB], bf16, tag=f"s{k}{p}{par}")
                    col = ds(jb * UB, UB) if p == 0 else ds(
                        (NBLK - 1) * UB - jb * UB, UB
                    )
                    src = xT0[:, col] if l == 0 else xin[l][k, :, col]
                    nc.sync.dma_start(t_[:], src)
                    sl[(k, p)] = t_
            return sl

        def proj_mgroup(l, d, wt, sl, xpb, m):
            """One ATOMIC projection accumulation group (gate chunk m, all 16
            steps of a block at N=512), then bias-fused copies to SBUF bf16
            split across Vector and Scalar. Layer 0 has a single K chunk with
            the bias riding the ones-row, so its copies are bias-free."""
            wih_t, _, b_t = wt[d]
            ps = pj[d].tile([128, UB], f32, tag="pjps")
            if l == 0:
                p = 0 if d == 0 else 1
                slab3 = sl[(0, p)][:].rearrange("p (u b) -> p u b", b=B)
                rhs = slab3 if d == 0 else slab3[:, ::-1, :]
                nc.tensor.matmul(
                    ps[:],
                    wih_t[:, m * 128 : (m + 1) * 128],
                    rhs,
                    start=True,
                    stop=True,
                )
            else:
                for k in range(4):
                    straight = (d == 0) if k < 2 else (d == 1)
                    p = 0 if straight else 1
                    slab3 = sl[(k, p)][:].rearrange("p (u b) -> p u b", b=B)
                    rhs = slab3 if straight else slab3[:, ::-1, :]
                    nc.tensor.matmul(
                        ps[:],
                        wih_t[:, (k * 8 + m) * 128 : (k * 8 + m + 1) * 128],
                        rhs,
                        start=(k == 0),
                        stop=(k == 3),
                    )
            ps3 = ps[:].rearrange("p (u b) -> p u b", b=B)
            hu = U // 2
            for piece in range(2):
                dst = xpb[:, m, piece * hu : (piece + 1) * hu, :]
                srcp = ps3[:, piece * hu : (piece + 1) * hu, :]
                if l == 2 or (m + piece) % 2 == 0:
                    if l == 0:
                        nc.vector.tensor_copy(dst, srcp)
                    else:
                        nc.vector.tensor_scalar_add(
                            dst, srcp, b_t[:, m : m + 1]
                        )
                else:
                    if l == 0:
                        nc.scalar.copy(dst, srcp)
                    else:
                        nc.scalar.activation(
                            dst, srcp, AF.Identity, bias=b_t[:, m : m + 1]
                        )

        def step_group(l, st, wt, sl, xpb, s, par):
            """Open the 8 gate-chunk PSUM regions for stream st at step s
            with the xp identity injection, then the recurrent matmuls."""
            wih_t, whh_t, _ = wt[st.d]
            psB = rpsB[st.sid].tile([128, 8, st.bw], f32, tag="psB")
            hh = hhs[st.sid]
            sp = U - 1 if s == 0 else s - 1
            nc.tensor.matmul(
                psB[:],
                ident[:],
                xpb[st.d][par][:, :, s, st.b0 : st.b0 + st.bw],
                start=True,
                stop=False,
            )
            for m in _MORD:
                for k in range(2):
                    nc.tensor.matmul(
                        psB[:, m, :],
                        whh_t[:, (k * 8 + m) * 128 : (k * 8 + m + 1) * 128],
                        hh[:, k, sp, :],
                        start=False,
                        stop=(k == 1),
                    )
            return psB

        def cell_chain(st, s, psB, fin=None):
            # psB holds a_i/2, a_f/2, a_o/2, a_g (weights prescaled); cc = 2c.
            w = st.bw
            cc = ccs[st.sid]
            gf = psB[:].rearrange("p m b -> p (m b)")
            t = gpool.tile([128, 8 * w], bf16, tag=f"t{st.sid}")
            nc.scalar.activation(t[:], gf[:], AF.Tanh)
            ti, tf = t[:, 0 : 2 * w], t[:, 2 * w : 4 * w]
            to, tg = t[:, 4 * w : 6 * w], t[:, 6 * w : 8 * w]
            m2 = tmp.tile([128, 2 * w], bf16, tag=f"tb{st.sid}")
            nc.vector.scalar_tensor_tensor(
                m2[:], ti, 1.0, tg, AOP.add, AOP.mult
            )  # (ti+1)*tg = 2ig
            m1 = tmp.tile([128, 2 * w], bf16, tag=f"ta{st.sid}")
            nc.vector.scalar_tensor_tensor(
                m1[:], tf, 1.0, cc[:], AOP.add, AOP.mult
            )  # (tf+1)*2c = 4fc
            nc.vector.scalar_tensor_tensor(
                cc[:], m1[:], 0.5, m2[:], AOP.mult, AOP.add
            )  # 2c' = 2fc + 2ig
            tcb = tmp.tile([128, 2 * w], bf16, tag=f"tc{st.sid}")
            nc.scalar.activation(tcb[:], cc[:], AF.Tanh, scale=0.5)
            hv = hhs[st.sid][:, :, s, :]
            to3 = to.rearrange("p (k b) -> p k b", b=w)
            tc3 = tcb[:].rearrange("p (k b) -> p k b", b=w)
            nc.vector.scalar_tensor_tensor(
                hv, to3, 1.0, tc3, AOP.add, AOP.mult
            )  # (to+1)*tanh(c') = 2h -> bf16
            if fin is not None:
                nc.vector.scalar_tensor_tensor(
                    fin[:, :, st.b0 : st.b0 + st.bw], to3, 1.0, tc3,
                    AOP.add, AOP.mult,
                )

        def rec_block(l, streams, dirs, wt, xpb, jb, par, prefetch, store,
                      fin=None):
            if prefetch:
                if l == 0:
                    # persistent L0 slab tiles (stable objects across the
                    # hardware loop) — prefetch block jb+1 into parity 1-par
                    for p in range(2):
                        col = ds((jb + 1) * UB, UB) if p == 0 else ds(
                            (NBLK - 2) * UB - jb * UB, UB
                        )
                        nc.sync.dma_start(
                            xsl[(p, 1 - par)][0 : I0 + 1, :], xT0[:, col]
                        )
                    sln = {(0, 0): xsl[(0, 1 - par)], (0, 1): xsl[(1, 1 - par)]}
                else:
                    sln = load_slab_set(l, jb + 1, 1 - par, dirs)
            for s in range(U):
                pss = {}
                for st in streams:
                    pss[st.sid] = step_group(l, st, wt, None, xpb, s, par)
                for st in streams:
                    f = (
                        fin
                        if (fin is not None and s == U - 1)
                        else None
                    )
                    cell_chain(st, s, pss[st.sid], fin=f)
                if prefetch:
                    if len(dirs) == 2:
                        d_, m_ = s % 2, s // 2
                    else:
                        d_, m_ = dirs[0], (s // 2 if s % 2 == 0 else None)
                    if m_ is not None:
                        proj_mgroup(l, d_, wt, sln, xpb[d_][1 - par], m_)
            if store:
                for st in streams:
                    for k in range(2):
                        nc.sync.dma_start(
                            xin[l + 1][2 * st.d + k, :, ds(jb * UB, UB)],
                            hhs[st.sid][:, k, :, :].rearrange(
                                "p u b -> p (u b)"
                            ),
                        )

        hhs, ccs, xsl = {}, {}, {}

        def rec_layer(l, wt, streams, store=True, fin=None):
            dirs = sorted({st.d for st in streams})
            for st in streams:
                hhist = state.tile([128, 2, U, st.bw], bf16, tag=f"h{st.sid}")
                cc = state.tile([128, 2 * st.bw], bf16, tag=f"c{st.sid}")
                nc.gpsimd.memset(hhist[:], 0.0)
                nc.gpsimd.memset(cc[:], 0.0)
                hhs[st.sid], ccs[st.sid] = hhist, cc
            xpb = {}
            for d in dirs:
                xpb[d] = {
                    0: state.tile(
                        [128, 8, U, B], bf16, tag=f"xpA{d}", name=f"xpA{d}"
                    ),
                    1: state.tile(
                        [128, 8, U, B], bf16, tag=f"xpB{d}", name=f"xpB{d}"
                    ),
                }
            # prologue: block 0 inputs + block-0 projection
            if l == 0:
                # persistent x slabs, K padded to 128 (zero rows 17..127)
                for p in range(2):
                    for q in range(2):
                        xsl[(p, q)] = state.tile(
                            [128, UB], bf16, tag=f"xs{p}{q}",
                            name=f"xs{p}{q}",
                        )
                        nc.gpsimd.memset(xsl[(p, q)][:], 0.0)
                    col = ds(0, UB) if p == 0 else ds((NBLK - 1) * UB, UB)
                    nc.sync.dma_start(
                        xsl[(p, 0)][0 : I0 + 1, :], xT0[:, col]
                    )
                sl0 = {(0, 0): xsl[(0, 0)], (0, 1): xsl[(1, 0)]}
            else:
                sl0 = load_slab_set(l, 0, 0, dirs)
            for d in dirs:
                for m in _MORD:
                    proj_mgroup(l, d, wt, sl0, xpb[d][0], m)
            args = (l, streams, dirs, wt, xpb)
            with tc.For_i(
                0, (NBLK - 8) // 8, 1, hint_engines=(mybir.EngineType.PE,)
            ) as jb8:
                for q in range(8):
                    rec_block(*args, 8 * jb8 + q, q % 2, True, store)
            for jb in range(NBLK - 8, NBLK - 1):
                rec_block(*args, jb, jb % 2, True, store)
            rec_block(*args, NBLK - 1, 1, False, store, fin=fin)

        # ---- layers ----
        full = lambda: [_Stream(0, 0, 0, B), _Stream(1, 1, 0, B)]
        wt = load_weights(0)
        rec_layer(0, wt, full())
        wt = load_weights(1)
        rec_layer(1, wt, full())
        wt = load_weights(2)
        hfin = state.tile([128, 2, B], f32, tag="hfin")
        halves = [_Stream(0, 0, 0, B // 2), _Stream(1, 0, B // 2, B // 2)]
        rec_layer(2, wt, halves, store=False, fin=hfin)
        nc.sync.dma_start(
            out_h2f[:], hfin[:].rearrange("p k b -> p (k b)")
        )

        # layer-2 reverse: only its first step (t = T-1) feeds the output.
        # h_prev = c_prev = 0 so gates = Wih_r . x2(T-1) + b and c = i*g.
        wih_t, _, _ = wt[1]
        b2 = wpool.tile([128, 8 * B], f32, tag="b2rb")
        nc.sync.dma_start(b2[:], b2rb[:])
        xs = {}
        for k in range(4):
            t_ = slabpool.tile([128, B], bf16, tag=f"l2r{k}")
            col = ds((NBLK - 1) * UB + (U - 1) * B, B) if k < 2 else ds(0, B)
            nc.sync.dma_start(t_[:], xin[2][k, :, col])
            xs[k] = t_
        psr = rpsB[1].tile([128, 8, B], f32, tag="psB")
        for m in range(8):
            for k in range(4):
                nc.tensor.matmul(
                    psr[:, m, :],
                    wih_t[:, (k * 8 + m) * 128 : (k * 8 + m + 1) * 128],
                    xs[k][:],
                    start=(k == 0),
                    stop=(k == 3),
                )
        g = gpool.tile([128, 8 * B], f32, tag="l2r_g")
        nc.vector.tensor_add(g[:], psr[:].rearrange("p m b -> p (m b)"), b2[:])
        t2 = gpool.tile([128, 8 * B], f32, tag="l2r_t")
        nc.scalar.activation(t2[:], g[:], AF.Tanh)
        cr = state.tile([128, 2 * B], f32, tag="l2r_c")
        nc.vector.scalar_tensor_tensor(
            cr[:], t2[:, 0 : 2 * B], 1.0, t2[:, 6 * B : 8 * B],
            AOP.add, AOP.mult,
        )  # 2c = (ti+1)*tg
        tcb = tmp.tile([128, 2 * B], f32, tag="l2r_tc")
        nc.scalar.activation(tcb[:], cr[:], AF.Tanh, scale=0.5)
        hr = state.tile([128, 2 * B], f32, tag="l2r_h")
        nc.vector.scalar_tensor_tensor(
            hr[:], t2[:, 4 * B : 6 * B], 1.0, tcb[:], AOP.add, AOP.mult
        )  # 2h
        nc.sync.dma_start(out_h2r[:], hr[:])


def _make_in_maps(inputs, T=TFULL, B=32, ncores=NCORES):
    x = np.asarray(inputs["x"], np.float32)
    shared = {}
    for l in range(3):
        for d, dn in enumerate("fr"):
            if l == 0:
                shared[f"wih{l}{dn}"] = _prep_wih0(
                    inputs[f"wih{l}{dn}"], inputs[f"b{l}{dn}"]
                )
            else:
                shared[f"wih{l}{dn}"] = _prep_wih(
                    inputs[f"wih{l}{dn}"], half=True
                )
                shared[f"b{l}{dn}"] = _prep_b(inputs[f"b{l}{dn}"])
            shared[f"whh{l}{dn}"] = _prep_wih(inputs[f"whh{l}{dn}"], half=True)
    shared["b2rb"] = np.ascontiguousarray(
        np.repeat(shared["b2r"], B, axis=1).astype(np.float32)
    )
    shared["ident"] = np.ascontiguousarray(np.eye(128, dtype=np_bf16))
    in_maps = []
    for ci in range(ncores):
        xs = x[ci * B : (ci + 1) * B, :T]  # [B, T, 16]
        xt = xs.transpose(2, 1, 0).reshape(I0, T * B)
        xt17 = np.vstack([xt, np.ones((1, T * B), np.float32)])
        m = dict(shared)
        m["xT0"] = np.ascontiguousarray(xt17.astype(np_bf16))
        in_maps.append(m)
    return in_maps


def _assemble(results, inputs, B=32):
    fcw = np.asarray(inputs["fcw"], np.float32)[0]
    fcb = float(np.asarray(inputs["fcb"], np.float32)[0])
    out = np.empty(len(results) * B, np.float32)
    for ci, r in enumerate(results):
        # device outputs are 2h — fold the 0.5 into the fc weights here
        h2f = np.concatenate([r["h2f"][:, :B], r["h2f"][:, B:]], axis=0)
        h2r = np.concatenate([r["h2r"][:, :B], r["h2r"][:, B:]], axis=0)
        out[ci * B : (ci + 1) * B] = (
            0.5 * (fcw[:256] @ h2f + fcw[256:] @ h2r) + fcb
        )
    return out


def kernel(**inputs):
    nc = bacc.Bacc(
        "TRN2", target_bir_lowering=False, debug=False, num_devices=NCORES
    )
    build(nc)
    nc.compile()
    in_maps = _make_in_maps(inputs)
    trace = os.environ.get("KERNEL_TRACE", "0") == "1"
    res = bass_utils.run_bass_kernel_spmd(
        nc,
        in_maps,
        core_ids=list(range(NCORES)),
        trace=trace,
        tmpdir=os.environ.get("KERNEL_TRACE_DIR") if trace else None,
    )
    if trace and res.exec_time_ns is not None:
        print(f"HW exec time: {res.exec_time_ns} ns")
    return _assemble(res.results, inputs)


# revision 42
# speedup vs baseline: 1.1949x; 1.1949x over previous
"""Bass/Tile kernel for a 3-layer bidirectional LSTM classifier on 8 TRN2 cores.

Problem shapes (hardcoded): x [256, 512, 16], H=256, 3 BiLSTM layers, fc -> [256].

Strategy: data-parallel over batch (B=32 per core, no collectives). Per core,
each layer runs as concurrent "streams" whose per-step chains hide each other's
latency: layers 0/1 use (fwd, rev); layer 2 (only fwd feeds the output) splits
the batch into two half-streams. All state is transposed (h [H, B], gates
[4H, B]) so matmuls keep weights PE-stationary; gate rows are pre-permuted to
[i, i, f, f, o, o, g, g] chunk order.

Numerics: every sigmoid becomes tanh via sigma(x) = (tanh(x/2)+1)/2 — the
i,f,o gate rows are prescaled 0.5 host-side so ONE tanh activation covers all
8 gate chunks; the cell update is 4 fused scalar_tensor_tensor ops on DVE
(state kept as 2c / 2h in bf16, absorbed by scaling whh and the next layer's
wih by 0.5; the final fc absorbs the last 0.5 in numpy).

Projection: computed one 16-step block ahead at N=512 into PSUM (one ATOMIC
accumulation group per gate chunk — PSUM region groups MUST be strictly
sequential: opening several regions with separate start=True matmuls before
closing them corrupts accumulation state on HW; only a single multi-region
matmul, like the xp identity injection, may open many regions at once). A
bias-folding copy then moves PSUM -> SBUF bf16, split into two halves
alternating Vector/Scalar (Vector-only for layer 2, whose Scalar is loaded by
4 activations/step) so no single copy blocks a chain op for long. Layer 0
rides its bias on a ones-row of x through the projection (weights K-padded to
128 — uniform K keeps back-to-back matmuls at the ~64-cycle dispatch floor).
The per-step PSUM group holds the xp identity injection plus the two
recurrent matmuls per gate chunk; activations read gates straight from PSUM.
Layer-0 x slabs live in four persistent SBUF tiles (stable tile objects —
handing fresh pool tiles across hardware-loop iterations deadlocks the Tile
scheduler); the block loop is unrolled 8 deep inside tc.For_i to amortize the
~2-3us/iteration COMPARE_BRANCH cost."""

import os
from contextlib import ExitStack

import numpy as np
import ml_dtypes

import concourse.bass as bass
import concourse.mybir as mybir
import concourse.tile as tile
from concourse import bacc, bass_utils
from concourse.bass import ds

f32 = mybir.dt.float32
bf16 = mybir.dt.bfloat16
AF = mybir.ActivationFunctionType
AOP = mybir.AluOpType
np_bf16 = ml_dtypes.bfloat16

H = 256
NCORES = 8
BFULL = 256
TFULL = 512
I0 = 16

# gate chunk order i,i,f,f,o,o,g,g (PyTorch order in weights is i,f,g,o)
_PERM = np.concatenate(
    [np.arange(0, 512), np.arange(768, 1024), np.arange(512, 768)]
)
_MORD = (6, 7, 0, 1, 2, 3, 4, 5)
# tanh-trick prescale: i,f,o chunks 0.5 (perm order i,i,f,f,o,o,g,g)
_GATE_SCALE = np.repeat([0.5, 0.5, 0.5, 1.0], 256)[:, None]


def _prep_wih(w, half=False):
    """[1024, Din] -> stationary layout [min(Din,128), nk*1024], chunk (k, m)
    at cols k*1024 + m*128; lhsT[kk, m*128+mm] = w_perm[m*128+mm, k*128+kk]."""
    wr = np.asarray(w, np.float32)[_PERM] * _GATE_SCALE
    if half:
        wr = wr * 0.5
    din = wr.shape[1]
    if din <= 128:
        return np.ascontiguousarray(wr.T.astype(np_bf16))
    nk = din // 128
    out = np.empty((128, nk * 1024), np_bf16)
    for k in range(nk):
        out[:, k * 1024 : (k + 1) * 1024] = wr[:, k * 128 : (k + 1) * 128].T
    return out


def _prep_wih0(w, b):
    """Layer-0 stationary: [128, 1024] = [wih.T ; bias row ; zero pad] — K is
    padded to 128 so the fused matmuls pipeline with the K=128 recurrent ones
    (the x slab rows 17..127 are zeroed on device)."""
    wr = np.asarray(w, np.float32)[_PERM] * _GATE_SCALE  # [1024, 16]
    br = np.asarray(b, np.float32)[_PERM] * _GATE_SCALE[:, 0]  # [1024]
    out = np.zeros((128, 1024), np.float32)
    out[:I0] = wr.T
    out[I0] = br
    return np.ascontiguousarray(out.astype(np_bf16))


def _prep_b(b):
    bs = np.asarray(b, np.float32)[_PERM] * _GATE_SCALE[:, 0]
    return np.ascontiguousarray(bs.reshape(8, 128).T)


class _Stream:
    __slots__ = ("sid", "d", "b0", "bw")

    def __init__(self, sid, d, b0, bw):
        self.sid, self.d, self.b0, self.bw = sid, d, b0, bw


def build(nc, T=TFULL, B=32):
    """Emit the full per-core program into nc (a Bacc)."""
    TB = T * B
    U = 16
    NBLK = T // U
    UB = U * B

    xT0 = nc.dram_tensor("xT0", [I0 + 1, TB], bf16, kind="ExternalInput").ap()
    win = {}
    for l in range(3):
        for d, dn in enumerate("fr"):
            if l == 0:
                win[(l, d, "wih")] = nc.dram_tensor(
                    f"wih{l}{dn}", [128, 1024], bf16, kind="ExternalInput"
                ).ap()
            else:
                win[(l, d, "wih")] = nc.dram_tensor(
                    f"wih{l}{dn}", [128, 4096], bf16, kind="ExternalInput"
                ).ap()
                win[(l, d, "b")] = nc.dram_tensor(
                    f"b{l}{dn}", [128, 8], f32, kind="ExternalInput"
                ).ap()
            win[(l, d, "whh")] = nc.dram_tensor(
                f"whh{l}{dn}", [128, 2048], bf16, kind="ExternalInput"
            ).ap()
    b2rb = nc.dram_tensor("b2rb", [128, 8 * B], f32, kind="ExternalInput").ap()
    identin = nc.dram_tensor("ident", [128, 128], bf16, kind="ExternalInput").ap()
    out_h2f = nc.dram_tensor("h2f", [128, 2 * B], f32, kind="ExternalOutput").ap()
    out_h2r = nc.dram_tensor("h2r", [128, 2 * B], f32, kind="ExternalOutput").ap()

    with tile.TileContext(nc) as tc, ExitStack() as ctx:
        dram = ctx.enter_context(tc.tile_pool(name="dram", bufs=1, space="DRAM"))
        wpool = ctx.enter_context(tc.tile_pool(name="wts", bufs=1))
        slabpool = ctx.enter_context(tc.tile_pool(name="slab", bufs=1))
        pj = [
            ctx.enter_context(tc.tile_pool(name=f"pj{d}", bufs=2, space="PSUM"))
            for d in range(2)
        ]
        rpsB = [
            ctx.enter_context(tc.tile_pool(name=f"rps{d}", bufs=2, space="PSUM"))
            for d in range(2)
        ]
        gpool = ctx.enter_context(tc.tile_pool(name="g", bufs=3))
        state = ctx.enter_context(tc.tile_pool(name="st", bufs=1))
        tmp = ctx.enter_context(tc.tile_pool(name="tmp", bufs=3))

        xin = {
            1: dram.tile([4, 128, TB], bf16, tag="xin1", name="xin1"),
            2: dram.tile([4, 128, TB], bf16, tag="xin2", name="xin2"),
        }
        ident = wpool.tile([128, 128], bf16, tag="ident")
        nc.sync.dma_start(ident[:], identin[:])

        def load_weights(l):
            kp = 128
            kch = 1 if l == 0 else 4
            wt = {}
            for d in range(2):
                wih_t = wpool.tile([kp, kch * 1024], bf16, tag=f"wih{d}")
                nc.sync.dma_start(wih_t[:], win[(l, d, "wih")][:])
                whh_t = wpool.tile([128, 2048], bf16, tag=f"whh{d}")
                nc.sync.dma_start(whh_t[:], win[(l, d, "whh")][:])
                if l == 0:
                    b_t = None
                else:
                    b_t = wpool.tile([128, 8], f32, tag=f"b{d}")
                    nc.sync.dma_start(b_t[:], win[(l, d, "b")][:])
                wt[d] = (wih_t, whh_t, b_t)
            return wt

        def load_slab_set(l, jb, par, dirs):
            """Stage the input columns block jb needs (p=0 fwd block jb,
            p=1 the mirrored block NBLK-1-jb; layers >=1 always need both:
            the k=2,3 input channels are stored time-reversed)."""
            sl = {}
            kk = 1 if l == 0 else 4
            kp = I0 + 1 if l == 0 else 128
            for k in range(kk):
                for p in range(2):
                    t_ = slabpool.tile([kp, UB], bf16, tag=f"s{k}{p}{par}")
                    col = ds(jb * UB, UB) if p == 0 else ds(
                        (NBLK - 1) * UB - jb * UB, UB
                    )
                    src = xT0[:, col] if l == 0 else xin[l][k, :, col]
                    nc.sync.dma_start(t_[:], src)
                    sl[(k, p)] = t_
            return sl

        def proj_mgroup(l, d, wt, sl, xpb, m):
            """One ATOMIC projection accumulation group (gate chunk m, all 16
            steps of a block at N=512), then bias-fused copies to SBUF bf16
            split across Vector and Scalar. Layer 0 has a single K chunk with
            the bias riding the ones-row, so its copies are bias-free."""
            wih_t, _, b_t = wt[d]
            ps = pj[d].tile([128, UB], f32, tag="pjps")
            if l == 0:
                p = 0 if d == 0 else 1
                slab3 = sl[(0, p)][:].rearrange("p (u b) -> p u b", b=B)
                rhs = slab3 if d == 0 else slab3[:, ::-1, :]
                nc.tensor.matmul(
                    ps[:],
                    wih_t[:, m * 128 : (m + 1) * 128],
                    rhs,
                    start=True,
                    stop=True,
                )
            else:
                for k in range(4):
                    straight = (d == 0) if k < 2 else (d == 1)
                    p = 0 if straight else 1
                    slab3 = sl[(k, p)][:].rearrange("p (u b) -> p u b", b=B)
                    rhs = slab3 if straight else slab3[:, ::-1, :]
                    nc.tensor.matmul(
                        ps[:],
                        wih_t[:, (k * 8 + m) * 128 : (k * 8 + m + 1) * 128],
                        rhs,
                        start=(k == 0),
                        stop=(k == 3),
                    )
            ps3 = ps[:].rearrange("p (u b) -> p u b", b=B)
            hu = U // 2
            for piece in range(2):
                dst = xpb[:, m, piece * hu : (piece + 1) * hu, :]
                srcp = ps3[:, piece * hu : (piece + 1) * hu, :]
                if l == 2 or (m + piece) % 2 == 0:
                    if l == 0:
                        nc.vector.tensor_copy(dst, srcp)
                    else:
                        nc.vector.tensor_scalar_add(
                            dst, srcp, b_t[:, m : m + 1]
                        )
                else:
                    if l == 0:
                        nc.scalar.copy(dst, srcp)
                    else:
                        nc.scalar.activation(
                            dst, srcp, AF.Identity, bias=b_t[:, m : m + 1]
                        )

        def step_group(l, st, wt, sl, xpb, s, par):
            """Open the 8 gate-chunk PSUM regions for stream st at step s
            with the xp identity injection, then the recurrent matmuls."""
            wih_t, whh_t, _ = wt[st.d]
            psB = rpsB[st.sid].tile([128, 8, st.bw], f32, tag="psB")
            hh = hhs[st.sid]
            sp = U - 1 if s == 0 else s - 1
            nc.tensor.matmul(
                psB[:],
                ident[:],
                xpb[st.d][par][:, :, s, st.b0 : st.b0 + st.bw],
                start=True,
                stop=False,
            )
            for m in _MORD:
                for k in range(2):
                    nc.tensor.matmul(
                        psB[:, m, :],
                        whh_t[:, (k * 8 + m) * 128 : (k * 8 + m + 1) * 128],
                        hh[:, k, sp, :],
                        start=False,
                        stop=(k == 1),
                    )
            return psB

        def cell_chain(st, s, psB, fin=None):
            # psB holds a_i/2, a_f/2, a_o/2, a_g (weights prescaled); cc = 2c.
            w = st.bw
            cc = ccs[st.sid]
            gf = psB[:].rearrange("p m b -> p (m b)")
            t = gpool.tile([128, 8 * w], bf16, tag=f"t{st.sid}")
            nc.scalar.activation(t[:], gf[:], AF.Tanh)
            ti, tf = t[:, 0 : 2 * w], t[:, 2 * w : 4 * w]
            to, tg = t[:, 4 * w : 6 * w], t[:, 6 * w : 8 * w]
            m2 = tmp.tile([128, 2 * w], bf16, tag=f"tb{st.sid}")
            nc.vector.scalar_tensor_tensor(
                m2[:], ti, 1.0, tg, AOP.add, AOP.mult
            )  # (ti+1)*tg = 2ig
            m1 = tmp.tile([128, 2 * w], bf16, tag=f"ta{st.sid}")
            nc.vector.scalar_tensor_tensor(
                m1[:], tf, 1.0, cc[:], AOP.add, AOP.mult
            )  # (tf+1)*2c = 4fc
            nc.vector.scalar_tensor_tensor(
                cc[:], m1[:], 0.5, m2[:], AOP.mult, AOP.add
            )  # 2c' = 2fc + 2ig
            tcb = tmp.tile([128, 2 * w], bf16, tag=f"tc{st.sid}")
            nc.scalar.activation(tcb[:], cc[:], AF.Tanh, scale=0.5)
            hv = hhs[st.sid][:, :, s, :]
            to3 = to.rearrange("p (k b) -> p k b", b=w)
            tc3 = tcb[:].rearrange("p (k b) -> p k b", b=w)
            nc.vector.scalar_tensor_tensor(
                hv, to3, 1.0, tc3, AOP.add, AOP.mult
            )  # (to+1)*tanh(c') = 2h -> bf16
            if fin is not None:
                nc.vector.scalar_tensor_tensor(
                    fin[:, :, st.b0 : st.b0 + st.bw], to3, 1.0, tc3,
                    AOP.add, AOP.mult,
                )

        def rec_block(l, streams, dirs, wt, xpb, jb, par, prefetch, store,
                      fin=None):
            if prefetch:
                if l == 0:
                    # persistent L0 slab tiles (stable objects across the
                    # hardware loop) — prefetch block jb+1 into parity 1-par
                    for p in range(2):
                        col = ds((jb + 1) * UB, UB) if p == 0 else ds(
                            (NBLK - 2) * UB - jb * UB, UB
                        )
                        nc.sync.dma_start(
                            xsl[(p, 1 - par)][0 : I0 + 1, :], xT0[:, col]
                        )
                    sln = {(0, 0): xsl[(0, 1 - par)], (0, 1): xsl[(1, 1 - par)]}
                else:
                    sln = load_slab_set(l, jb + 1, 1 - par, dirs)
            for s in range(U):
                pss = {}
                for st in streams:
                    pss[st.sid] = step_group(l, st, wt, None, xpb, s, par)
                for st in streams:
                    f = (
                        fin
                        if (fin is not None and s == U - 1)
                        else None
                    )
                    cell_chain(st, s, pss[st.sid], fin=f)
                if prefetch:
                    if len(dirs) == 2:
                        d_, m_ = s % 2, s // 2
                    else:
                        d_, m_ = dirs[0], (s // 2 if s % 2 == 0 else None)
                    if m_ is not None:
                        proj_mgroup(l, d_, wt, sln, xpb[d_][1 - par], m_)
            if store:
                for st in streams:
                    for k in range(2):
                        nc.sync.dma_start(
                            xin[l + 1][2 * st.d + k, :, ds(jb * UB, UB)],
                            hhs[st.sid][:, k, :, :].rearrange(
                                "p u b -> p (u b)"
                            ),
                        )

        hhs, ccs, xsl = {}, {}, {}

        def rec_layer(l, wt, streams, store=True, fin=None):
            dirs = sorted({st.d for st in streams})
            for st in streams:
                hhist = state.tile([128, 2, U, st.bw], bf16, tag=f"h{st.sid}")
                cc = state.tile([128, 2 * st.bw], bf16, tag=f"c{st.sid}")
                nc.gpsimd.memset(hhist[:], 0.0)
                nc.gpsimd.memset(cc[:], 0.0)
                hhs[st.sid], ccs[st.sid] = hhist, cc
            xpb = {}
            for d in dirs:
                xpb[d] = {
                    0: state.tile(
                        [128, 8, U, B], bf16, tag=f"xpA{d}", name=f"xpA{d}"
                    ),
                    1: state.tile(
                        [128, 8, U, B], bf16, tag=f"xpB{d}", name=f"xpB{d}"
                    ),
                }
            # prologue: block 0 inputs + block-0 projection
            if l == 0:
                # persistent x slabs, K padded to 128 (zero rows 17..127)
                for p in range(2):
                    for q in range(2):
                        xsl[(p, q)] = state.tile(
                            [128, UB], bf16, tag=f"xs{p}{q}",
                            name=f"xs{p}{q}",
                        )
                        nc.gpsimd.memset(xsl[(p, q)][:], 0.0)
                    col = ds(0, UB) if p == 0 else ds((NBLK - 1) * UB, UB)
                    nc.sync.dma_start(
                        xsl[(p, 0)][0 : I0 + 1, :], xT0[:, col]
                    )
                sl0 = {(0, 0): xsl[(0, 0)], (0, 1): xsl[(1, 0)]}
            else:
                sl0 = load_slab_set(l, 0, 0, dirs)
            for d in dirs:
                for m in _MORD:
                    proj_mgroup(l, d, wt, sl0, xpb[d][0], m)
            args = (l, streams, dirs, wt, xpb)
            with tc.For_i(
                0, (NBLK - 8) // 8, 1, hint_engines=(mybir.EngineType.PE,)
            ) as jb8:
                for q in range(8):
                    rec_block(*args, 8 * jb8 + q, q % 2, True, store)
            for jb in range(NBLK - 8, NBLK - 1):
                rec_block(*args, jb, jb % 2, True, store)
            rec_block(*args, NBLK - 1, 1, False, store, fin=fin)

        # ---- layers ----
        full = lambda: [_Stream(0, 0, 0, B), _Stream(1, 1, 0, B)]
        wt = load_weights(0)
        rec_layer(0, wt, full())
        wt = load_weights(1)
        rec_layer(1, wt, full())
        wt = load_weights(2)
        hfin = state.tile([128, 2, B], f32, tag="hfin")
        halves = [_Stream(0, 0, 0, B // 2), _Stream(1, 0, B // 2, B // 2)]
        rec_layer(2, wt, halves, store=False, fin=hfin)
        nc.sync.dma_start(
            out_h2f[:], hfin[:].rearrange("p k b -> p (k b)")
        )

        # layer-2 reverse: only its first step (t = T-1) feeds the output.
        # h_prev = c_prev = 0 so gates = Wih_r . x2(T-1) + b and c = i*g.
        wih_t, _, _ = wt[1]
        b2 = wpool.tile([128, 8 * B], f32, tag="b2rb")
        nc.sync.dma_start(b2[:], b2rb[:])
        xs = {}
        for k in range(4):
            t_ = slabpool.tile([128, B], bf16, tag=f"l2r{k}")
            col = ds((NBLK - 1) * UB + (U - 1) * B, B) if k < 2 else ds(0, B)
            nc.sync.dma_start(t_[:], xin[2][k, :, col])
            xs[k] = t_
        psr = rpsB[1].tile([128, 8, B], f32, tag="psB")
        for m in range(8):
            for k in range(4):
                nc.tensor.matmul(
                    psr[:, m, :],
                    wih_t[:, (k * 8 + m) * 128 : (k * 8 + m + 1) * 128],
                    xs[k][:],
                    start=(k == 0),
                    stop=(k == 3),
                )
        g = gpool.tile([128, 8 * B], f32, tag="l2r_g")
        nc.vector.tensor_add(g[:], psr[:].rearrange("p m b -> p (m b)"), b2[:])
        t2 = gpool.tile([128, 8 * B], f32, tag="l2r_t")
        nc.scalar.activation(t2[:], g[:], AF.Tanh)
        cr = state.tile([128, 2 * B], f32, tag="l2r_c")
        nc.vector.scalar_tensor_tensor(
            cr[:], t2[:, 0 : 2 * B], 1.0, t2[:, 6 * B : 8 * B],
            AOP.add, AOP.mult,
        )  # 2c = (ti+1)*tg
        tcb = tmp.tile([128, 2 * B], f32, tag="l2r_tc")
        nc.scalar.activation(tcb[:], cr[:], AF.Tanh, scale=0.5)
        hr = state.tile([128, 2 * B], f32, tag="l2r_h")
        nc.vector.scalar_tensor_tensor(
            hr[:], t2[:, 4 * B : 6 * B], 1.0, tcb[:], AOP.add, AOP.mult
        )  # 2h
        nc.sync.dma_start(out_h2r[:], hr[:])


def _make_in_maps(inputs, T=TFULL, B=32, ncores=NCORES):
    x = np.asarray(inputs["x"], np.float32)
    shared = {}
    for l in range(3):
        for d, dn in enumerate("fr"):
            if l == 0:
                shared[f"wih{l}{dn}"] = _prep_wih0(
                    inputs[f"wih{l}{dn}"], inputs[f"b{l}{dn}"]
                )
            else:
                shared[f"wih{l}{dn}"] = _prep_wih(
                    inputs[f"wih{l}{dn}"], half=True
                )
                shared[f"b{l}{dn}"] = _prep_b(inputs[f"b{l}{dn}"])
            shared[f"whh{l}{dn}"] = _prep_wih(inputs[f"whh{l}{dn}"], half=True)
    shared["b2rb"] = np.ascontiguousarray(
        np.repeat(shared["b2r"], B, axis=1).astype(np.float32)
    )
    shared["ident"] = np.ascontiguousarray(np.eye(128, dtype=np_bf16))
    in_maps = []
    for ci in range(ncores):
        xs = x[ci * B : (ci + 1) * B, :T]  # [B, T, 16]
        xt = xs.transpose(2, 1, 0).reshape(I0, T * B)
        xt17 = np.vstack([xt, np.ones((1, T * B), np.float32)])
        m = dict(shared)
        m["xT0"] = np.ascontiguousarray(xt17.astype(np_bf16))
        in_maps.append(m)
    return in_maps


def _assemble(results, inputs, B=32):
    fcw = np.asarray(inputs["fcw"], np.float32)[0]
    fcb = float(np.asarray(inputs["fcb"], np.float32)[0])
    out = np.empty(len(results) * B, np.float32)
    for ci, r in enumerate(results):
        # device outputs are 2h — fold the 0.5 into the fc weights here
        h2f = np.concatenate([r["h2f"][:, :B], r["h2f"][:, B:]], axis=0)
        h2r = np.concatenate([r["h2r"][:, :B], r["h2r"][:, B:]], axis=0)
        out[ci * B : (ci + 1) * B] = (
            0.5 * (fcw[:256] @ h2f + fcw[256:] @ h2r) + fcb
        )
    return out


def kernel(**inputs):
    nc = bacc.Bacc(
        "TRN2", target_bir_lowering=False, debug=False, num_devices=NCORES
    )
    build(nc)
    nc.compile()
    in_maps = _make_in_maps(inputs)
    trace = os.environ.get("KERNEL_TRACE", "0") == "1"
    res = bass_utils.run_bass_kernel_spmd(
        nc,
        in_maps,
        core_ids=list(range(NCORES)),
        trace=trace,
        tmpdir=os.environ.get("KERNEL_TRACE_DIR") if trace else None,
    )
    if trace and res.exec_time_ns is not None:
        print(f"HW exec time: {res.exec_time_ns} ns")
    return _assemble(res.results, inputs)
